# revision 36
# baseline (speedup 1.0000x reference)
"""FFT-Conv2d on Trainium2 (Bass/Tile): hybrid H-DFT + direct W-correlation.

Math: the reference's rfft2/einsum/irfft2 pipeline is a *valid* 2-D
cross-correlation plus bias:

    out[b, d, i, j] = sum_{c,u,v} signal[b, c, i+u, j+v] * weight[d, c, u, v]
                      + bias[d]

with signal [16, 32, 256, 256], weight [32, 32, 31, 31] -> out [16, 32, 226, 226].

Default algorithm (ALGO="fft", data-parallel, 2 images/core x 8 cores):
transform H only (256-pt real DFT as dense PE matmuls) so the H-taps
become pointwise across 256 spectral "planes" (re/im parts); correlation
along W stays direct (31 taps).  Per core: 2240 matmuls vs 14464 for the
all-direct kernel (ALGO="direct", kept below).
  A) H-DFT: stationary = plane-permuted DFT matrix, rhs = signal
     [h, (c, w)]; psum accumulates 2 k-chunks; a scatter DMA re-lays
     planes into 64 group tiles [(slot, c)=128, b, w] (4 planes/group,
     complex pairs).
  B) per group: 31 taps of [128x128] @ [128, 2b, 229] accumulate the
     block-diagonal complex product in one PSUM bank.  The weight table
     (31 x 64 distinct stationaries, 32.5 MB fp16) streams from HBM as
     two 64x64 diagonal blocks into persistent SBUF tiles whose zero
     corners are memset once — weight DMA, not PE, bounds this stage.
     Bias rides the DC plane (out_h[0] += 256*bias; invF[:, 0] = 1/256).
  C) inverse H-DFT (crop to 226 rows) + store.
Engine-queue discipline (head-of-queue waits serialize a queue): SP does
only prefetch-style DMAs (signal, weights); ACT gets the late-waiting
B->C scatters; GPSIMD gets A-scatters + output stores; DVE does all psum
evacuations.  PSUM banks lean toward stage B (1/6/1); weight prefetch is
6 buffers deep with the two blocks on separate queues.  Measured: ~1.14 ms
HW exec, rel err 4.5e-4 (vs 3.47 ms for the direct kernel).
"""

import os
import sys

import numpy as np

for _p in ("/opt/trn_rl_repo",):
    if _p not in sys.path and os.path.isdir(_p):
        sys.path.insert(0, _p)

import concourse.bacc as bacc
import concourse.mybir as mybir
import concourse.tile as tile
from concourse.bass_utils import run_bass_kernel_spmd

# Problem constants (hardcoded per harness contract).
B, C, H, W = 16, 32, 256, 256
D, KH = 32, 31
TH = TW = 226
NCORES = 8
BPC = B // NCORES  # batches per core
HALO = 30          # extra sigrep rows below a tile (28 group offset + 2 wrap)


def _row_tiles():
    """Output-row tiles (start, nrows); nrows even."""
    r = int(os.environ.get("FFTCONV_R", "38"))
    tiles, i0 = [], 0
    while i0 < TH:
        n = min(r, TH - i0)
        assert n % 2 == 0
        tiles.append((i0, n))
        i0 += n
    return tiles


ROW_TILES = _row_tiles()

# key -> (weight dtype, signal dtype, use 3-D two-row rhs AP of width 229)
# float32r requires a 2-D (flat 512) moving AP; 16-bit dtypes can use the
# narrower 3-D AP (458 streamed columns instead of 512).
_DT_CONFIGS = {
    "f32r": (mybir.dt.float32r, mybir.dt.float32r, False),
    "f16": (mybir.dt.float16, mybir.dt.float16, True),
    "f16flat": (mybir.dt.float16, mybir.dt.float16, False),
    "bf16": (mybir.dt.bfloat16, mybir.dt.bfloat16, True),
    "f32": (mybir.dt.float32, mybir.dt.float32, True),
}
# f16 measured fastest on HW (one LDWEIGHTS per matmul is unavoidable with
# this toolchain; fp16 halves the weight-load and gets FWL).  rel err vs the
# fp32 FFT reference ~2.8e-4; use FFTCONV_DT=f32r for ~1.4e-4 at +15% time.
DT_KEY = os.environ.get("FFTCONV_DT", "f16")
# fp8 DoubleRow for the last FP8G7 groups of 4 kernel rows (1 -> rows 28..31,
# 2 -> rows 24..31): 0.5 cyc/row on those 8*FP8G7 matmuls per row-pair.
# Weight is packed hi+lo e4m3 (scaled x32 to clear the e4m3 subnormal hole at
# |w|~0.02) in the two DR k-subtiles, signal is bare e4m3; the groups
# accumulate in their own PSUM bank, merged x(1/32) in the epilogue.  Measured
# rel err: 7.9e-3 at FP8G7=1, ~1.2e-2 at 2 (tol 2e-2).
FP8G7 = int(os.environ.get("FFTCONV_FP8G7", "0"))
# V2: instead of hi/lo weight splitting, the two DR k-subtiles carry two
# DIFFERENT 2-row tap groups at 64 partitions (u2, c): one DR matmul covers
# kernel rows 28..31 with a 128-row stationary load (same ldweights cost as
# fp16).  Weights bare e4m3 (x32): rel err ~1.4e-2.
FP8V2 = int(os.environ.get("FFTCONV_FP8V2", "0"))
if FP8V2:
    FP8G7 = 1
W8SCALE = 32.0
FP8_ROW0 = 32 - 4 * FP8G7  # first kernel row handled in fp8
# Row-pairs per weight-stationary wave (= PSUM banks cycled).  Measured on
# HW: WAVE=1 (bank-stable, weights reloaded per matmul) beats WAVE=8
# (stationary reuse but per-matmul PSUM bank switching stalls the PE).
WAVE = int(os.environ.get("FFTCONV_WAVE", "1"))


def _np_dt(dt_mm):
    return mybir.dt.np(dt_mm)


def build_program(dt_key: str = DT_KEY, repeat: int = 1):
    """Build the SPMD Bass program (one NeuronCore's slice: BPC batches)."""
    wt_dt, dt_mm, use3d = _DT_CONFIGS[dt_key]
    f32 = mybir.dt.float32
    NJ = 229
    # 0 = self-loading matmuls; 1 = explicit ldweights + ldweights=False
    # flags (needs walrus --enable-ldw-opt=false); 2 = ldweights=False flags
    # only (needs --enable-ldw-opt=true, which rejects explicit InstLdweights).
    # Standalone ldweights is rejected for 4-byte dtypes (fp32/fp32r).
    LDW_SHARE = (
        int(os.environ.get("FFTCONV_LDW_SHARE", "0"))
        if wt_dt in (mybir.dt.float16, mybir.dt.bfloat16)
        else 0
    )
    nc = bacc.Bacc(
        "TRN2",
        target_bir_lowering=False,
        debug=False,
        enable_asserts=False,
        num_devices=NCORES,
    )
    fp8 = mybir.dt.float8e4
    sig_d = nc.dram_tensor("signal", [BPC, C, H, W], dt_mm, kind="ExternalInput")
    wt_d = nc.dram_tensor("wT", [128, 8, 8, 128], wt_dt, kind="ExternalInput")
    bias_d = nc.dram_tensor("bias", [D, 1], f32, kind="ExternalInput")
    if FP8G7:
        sig8_d = nc.dram_tensor("sig8", [BPC, C, H, W], fp8, kind="ExternalInput")
        wt8_d = nc.dram_tensor(
            "wT8",
            [64, 2, 8, 128] if FP8V2 else [128, FP8G7, 2, 8, 128],
            fp8,
            kind="ExternalInput",
        )
    out_d = nc.dram_tensor("out", [BPC, D, TH, TW], f32, kind="ExternalOutput")

    SIG_BUFS = int(os.environ.get("FFTCONV_SIG_BUFS", "2" if FP8G7 else "3"))
    TMP_BUFS = int(os.environ.get("FFTCONV_TMP_BUFS", "2" if FP8G7 else "4"))
    OUT_BUFS = int(os.environ.get("FFTCONV_OUT_BUFS", "4" if FP8G7 else "8"))
    with tile.TileContext(nc) as tc:
        with (
            tc.tile_pool(name="const", bufs=1) as const_pool,
            tc.tile_pool(name="sig", bufs=SIG_BUFS) as sig_pool,
            tc.tile_pool(name="psum", bufs=4 if FP8G7 else 8, space="PSUM") as psum_pool,
            tc.tile_pool(name="tmp", bufs=TMP_BUFS) as tmp_pool,
            tc.tile_pool(name="outb", bufs=OUT_BUFS) as out_pool,
        ):
            wt = const_pool.tile([128, 8, 8, 128], wt_dt)
            nc.sync.dma_start(wt[:, :, :, :], wt_d[:, :, :, :])
            bias_t = const_pool.tile([D, 1], f32)
            nc.sync.dma_start(bias_t[:, :], bias_d[:, :])
            if FP8V2:
                wt8 = const_pool.tile([64, 2, 8, 128], fp8)
                nc.sync.dma_start(wt8[:, :, :, :], wt8_d[:, :, :, :])
            elif FP8G7:
                wt8 = const_pool.tile([128, FP8G7, 2, 8, 128], fp8)
                nc.sync.dma_start(wt8[:, :, :, :, :], wt8_d[:, :, :, :, :])

            for b in [bb for _ in range(repeat) for bb in range(BPC)]:
                for i0, R in _row_tiles():
                    # Signal rows stored CONTIGUOUSLY at pitch 256 (= W) so a
                    # two-row matmul rhs is one flat 512 span (float32r
                    # requires a 2-D moving AP).  Column overruns wrap to the
                    # next row but only land on zero-weight taps / unread
                    # psum columns.
                    r_tot = R + HALO
                    srep = sig_pool.tile([128, r_tot * W], dt_mm, tag="srep")
                    srep3 = srep[:].rearrange("p (r w) -> p r w", w=W)
                    for u in range(4):
                        rows = min(r_tot, H - (i0 + u))
                        nc.sync.dma_start(
                            srep3[u * 32 : (u + 1) * 32, 0:rows, :],
                            sig_d[b, :, i0 + u : i0 + u + rows, :],
                        )
                        if rows < r_tot:
                            nc.vector.memset(
                                srep3[u * 32 : (u + 1) * 32, rows:r_tot, :].bitcast(
                                    mybir.dt.float32
                                ),
                                0.0,
                            )
                    if FP8V2:
                        # partition (u2*32+c) holds sig8 row i0+28+2t+u2+r:
                        # DR k-subtile t covers kernel rows 28+2t..29+2t.
                        r8_tot = R + 1
                        srep8 = sig_pool.tile([64, 2, r8_tot * W], fp8, tag="srep8")
                        srep8_4 = srep8[:].rearrange("p t (r w) -> p t r w", w=W)
                        for u in range(2):
                            for t in range(2):
                                base = i0 + 28 + 2 * t + u
                                rows8 = max(0, min(r8_tot, H - base))
                                if rows8:
                                    nc.sync.dma_start(
                                        srep8_4[u * 32 : (u + 1) * 32, t, 0:rows8, :],
                                        sig8_d[b, :, base : base + rows8, :],
                                    )
                                if rows8 < r8_tot:
                                    nc.vector.memset(
                                        srep8_4[
                                            u * 32 : (u + 1) * 32, t, rows8:r8_tot, :
                                        ].bitcast(mybir.dt.float32),
                                        0.0,
                                    )
                    elif FP8G7:
                        # fp8 replica for kernel rows FP8_ROW0..31: partition
                        # (u*32+c) holds sig8 row i0+FP8_ROW0+u+r; both DR
                        # k-subtiles carry the same signal (weight is split
                        # hi/lo across them).
                        r8_tot = R + 4 * FP8G7 - 3
                        srep8 = sig_pool.tile([128, 2, r8_tot * W], fp8, tag="srep8")
                        srep8_4 = srep8[:].rearrange("p t (r w) -> p t r w", w=W)
                        for u in range(4):
                            base = i0 + FP8_ROW0 + u
                            rows8 = max(0, min(r8_tot, H - base))
                            for t in range(2):
                                if rows8:
                                    nc.sync.dma_start(
                                        srep8_4[u * 32 : (u + 1) * 32, t, 0:rows8, :],
                                        sig8_d[b, :, base : base + rows8, :],
                                    )
                                if rows8 < r8_tot:
                                    nc.vector.memset(
                                        srep8_4[
                                            u * 32 : (u + 1) * 32, t, rows8:r8_tot, :
                                        ].bitcast(mybir.dt.float32),
                                        0.0,
                                    )
                    all_rps = list(range(R // 2))
                    for w0 in range(0, len(all_rps), WAVE):
                        wave = all_rps[w0 : w0 + WAVE]
                        # Weight-stationary: each (g, vb) lhsT streams all
                        # row-pairs of the wave (distinct PSUM banks) before
                        # the next weight load.
                        ps3s = []
                        ps8s = []
                        for rp in wave:
                            if use3d:
                                ps_t = psum_pool.tile([128, 2, NJ], f32, tag="ps")
                                ps3s.append(ps_t)
                            else:
                                ps_t = psum_pool.tile([128, 2 * W], f32, tag="ps")
                                ps3s.append(ps_t[:].rearrange("p (r w) -> p r w", w=W))
                            if FP8G7:
                                ps8_t = psum_pool.tile([128, 2 * W], f32, tag="ps8")
                                ps8s.append(ps8_t[:].rearrange("p (r w) -> p r w", w=W))
                        NG = 8 - FP8G7
                        for g in range(NG):
                            for vb in range(8):
                                # One explicit weight load per (g, vb); the
                                # wave's matmuls reuse the stationary operand
                                # (ldweights=False skips the per-matmul load
                                # walrus would otherwise emit).
                                if LDW_SHARE == 1 and len(wave) > 1:
                                    nc.tensor.ldweights(wt[:, g, vb, :])
                                for j, rp in enumerate(wave):
                                    off = (2 * rp + 4 * g) * W + 4 * vb
                                    ps3 = ps3s[j]
                                    if use3d:
                                        rhs = srep[:, off : off + 2 * W].rearrange(
                                            "p (r w) -> p r w", w=W
                                        )[:, :, 0:NJ]
                                        out_ap = ps3[:, :, :]
                                    else:
                                        rhs = srep[:, off : off + 2 * W]
                                        out_ap = ps3.rearrange("p r w -> p (r w)")
                                    mm = nc.tensor.matmul(
                                        out_ap,
                                        lhsT=wt[:, g, vb, :],
                                        rhs=rhs,
                                        start=(g == 0 and vb == 0),
                                        stop=(g == NG - 1 and vb == 7),
                                    )
                                    if LDW_SHARE and len(wave) > 1 and j > 0:
                                        mm.ins.ldweights = False
                                    elif LDW_SHARE == 1 and len(wave) > 1:
                                        # explicit ldweights above covers it
                                        mm.ins.ldweights = False
                        if FP8V2:
                            for vb in range(8):
                                for j, rp in enumerate(wave):
                                    off8 = 2 * rp * W + 4 * vb
                                    nc.tensor.matmul(
                                        ps8s[j].rearrange("p r w -> p (r w)"),
                                        lhsT=wt8[:, :, vb, :],
                                        rhs=srep8[:, :, off8 : off8 + 2 * W],
                                        start=(vb == 0),
                                        stop=(vb == 7),
                                        perf_mode=mybir.MatmulPerfMode.DoubleRow,
                                    )
                        elif FP8G7:
                            for g8 in range(FP8G7):
                                for vb in range(8):
                                    for j, rp in enumerate(wave):
                                        off8 = (2 * rp + 4 * g8) * W + 4 * vb
                                        nc.tensor.matmul(
                                            ps8s[j].rearrange("p r w -> p (r w)"),
                                            lhsT=wt8[:, g8, :, vb, :],
                                            rhs=srep8[:, :, off8 : off8 + 2 * W],
                                            start=(g8 == 0 and vb == 0),
                                            stop=(g8 == FP8G7 - 1 and vb == 7),
                                            perf_mode=mybir.MatmulPerfMode.DoubleRow,
                                        )
                        for j, rp in enumerate(wave):
                            i = i0 + 2 * rp
                            ps3 = ps3s[j]
                            # One PSUM operand per instruction (HW: single DVE
                            # PSUM read port).  ACT folds in the bias.
                            t0 = tmp_pool.tile([D, 2, TW], f32, tag="t0")
                            t1 = tmp_pool.tile([D, 2, TW], f32, tag="t1")
                            t2 = tmp_pool.tile([D, 2, TW], f32, tag="t2")
                            ob = out_pool.tile([D, 2, TW], f32, tag="ob")
                            nc.scalar.activation(
                                t0[:, :, :],
                                ps3[0:32, :, 0:226],
                                mybir.ActivationFunctionType.Identity,
                                bias=bias_t[:, :],
                            )
                            nc.vector.tensor_add(t1[:, :, :], t0[:, :, :], ps3[32:64, :, 1:227])
                            nc.vector.tensor_add(t2[:, :, :], t1[:, :, :], ps3[64:96, :, 2:228])
                            if not FP8G7:
                                nc.vector.tensor_add(ob[:, :, :], t2[:, :, :], ps3[96:128, :, 3:229])
                            else:
                                ps8 = ps8s[j]
                                t3 = tmp_pool.tile([D, 2, TW], f32, tag="t3")
                                q0 = tmp_pool.tile([D, 2, TW], f32, tag="q0")
                                q1 = tmp_pool.tile([D, 2, TW], f32, tag="q1")
                                q2 = tmp_pool.tile([D, 2, TW], f32, tag="q2")
                                q3 = tmp_pool.tile([D, 2, TW], f32, tag="q3")
                                q4 = tmp_pool.tile([D, 2, TW], f32, tag="q4")
                                nc.vector.tensor_add(t3[:, :, :], t2[:, :, :], ps3[96:128, :, 3:229])
                                nc.scalar.activation(
                                    q0[:, :, :],
                                    ps8[0:32, :, 0:226],
                                    mybir.ActivationFunctionType.Identity,
                                )
                                nc.vector.tensor_add(q1[:, :, :], q0[:, :, :], ps8[32:64, :, 1:227])
                                nc.vector.tensor_add(q2[:, :, :], q1[:, :, :], ps8[64:96, :, 2:228])
                                nc.vector.tensor_add(q3[:, :, :], q2[:, :, :], ps8[96:128, :, 3:229])
                                nc.scalar.activation(
                                    q4[:, :, :],
                                    q3[:, :, :],
                                    mybir.ActivationFunctionType.Identity,
                                    scale=1.0 / W8SCALE,
                                )
                                nc.vector.tensor_add(ob[:, :, :], t3[:, :, :], q4[:, :, :])
                            nc.sync.dma_start(out_d[b, :, i : i + 2, :], ob[:, :, :])
    nc.compile()
    # Off by default: only useful with WAVE>1 weight-stationary ordering,
    # which measured slower on HW (PSUM bank cycling).
    if int(os.environ.get("FFTCONV_LDW_DEDUP", "0")):
        bir = _dedupe_ldweights_json(nc.to_json_bytes())
        nc.to_json_bytes = lambda: bir  # instance override; cached bytes
    return nc


def _dedupe_ldweights_json(bir: bytes) -> bytes:
    """Drop PE Ldweights whose stationary operand is already loaded.

    tile_legalize splits every Matmult into Ldweights + Matmult(ldweights
    =false); with weight-stationary waves most loads are redundant reloads
    of the identical operand (measured ~107 ns each, serialized with the
    matmul stream).  Walrus's own dedupe (--enable-ldw-opt) is disabled in
    this toolchain, so do it on the serialized BIR: remove a Ldweights if
    the previous PE array load had the same operands/flags, carrying its
    semaphore waits/updates onto the next PE instruction.
    """
    import json as _json

    j = _json.loads(bir)
    removed = 0
    for fn in j.get("functions", []):
        for blk in fn.get("blocks", []):
            ins_l = blk.get("instructions")
            if not ins_l:
                continue
            out = []
            cur_sig = None
            for inst in ins_l:
                if inst.get("engine") != "PE":
                    out.append(inst)
                    continue
                op = inst.get("opcode")
                if op == "Ldweights":
                    sig = (
                        _json.dumps(inst.get("ins"), sort_keys=True),
                        inst.get("is_transpose"),
                        str(inst.get("perf_mode")),
                        str(inst.get("tile_position")),
                        str(inst.get("tile_size")),
                    )
                    si = inst.get("sync_info") or {}
                    if (
                        sig == cur_sig
                        and not (si.get("on_wait") or si.get("on_update"))
                    ):
                        # bare redundant reload: safe to drop (a Matmult can
                        # hold at most one ISA wait, so loads carrying sync
                        # stay).
                        removed += 1
                        continue
                    cur_sig = sig
                elif op == "Matmult":
                    if inst.get("ldweights") is not False:
                        cur_sig = None  # self-loading matmul replaces stationary
                elif op == "EventSemaphore":
                    pass  # pure semaphore op, array state unaffected
                else:
                    cur_sig = None  # Drain / branch: conservative reset
                out.append(inst)
            blk["instructions"] = out
    if removed:
        sys.stderr.write(f"[kernel] deduped {removed} redundant Ldweights\n")
    return _json.dumps(j).encode()


def pack_weights(weight: np.ndarray, np_dt) -> np.ndarray:
    """weight [D, C, 31, 31] -> lhsT table [128, 8, 8, 128].

    wT[(u_idx*32 + c), g, vb, (s*32 + d)] = weight[d, c, 4g+u_idx, 4vb+s],
    zero where 4g+u_idx > 30 or 4vb+s > 30.
    """
    w = np.zeros((D, C, 32, 32), np.float32)
    w[:, :, :KH, :KH] = weight.astype(np.float32)
    # -> [u_idx, c, g, vb, s, d]
    wt = w.reshape(D, C, 8, 4, 8, 4).transpose(3, 1, 2, 4, 5, 0)
    wt = wt.reshape(4 * C, 8, 8, 4 * D)
    return np.ascontiguousarray(wt.astype(np_dt))


def pack_weights8_v2(weight: np.ndarray) -> np.ndarray:
    """weight [D, C, 31, 31] -> DR lhsT [64, 2, 8, 128] bare e4m3.

    wT8[(u2*32 + c), t, vb, (s*32 + d)] = e4m3(W8SCALE*weight[d, c, 28+2t+u2, 4vb+s]),
    zero outside the kernel (row 31 and col 31).
    """
    np8 = _np_dt(mybir.dt.float8e4)
    w = np.zeros((D, C, 4, 32), np.float32)
    w[:, :, :3, :KH] = weight[:, :, 28:31, :].astype(np.float32) * W8SCALE
    # [D, C, (t,u2), vb, s] -> [u2, c, t, vb, s, d]
    tbl = w.reshape(D, C, 2, 2, 8, 4).transpose(3, 1, 2, 4, 5, 0)
    tbl = tbl.reshape(2 * C, 2, 8, 4 * D)
    return np.ascontiguousarray(tbl.astype(np8))


def pack_weights8(weight: np.ndarray) -> np.ndarray:
    """weight [D, C, 31, 31] -> DR lhsT table [128, FP8G7, 2, 8, 128] e4m3.

    wT8[(u*32 + c), g8, t, vb, (s*32 + d)] = hi/lo (t=0/1) of
    W8SCALE*weight[d, c, FP8_ROW0+4*g8+u, 4vb+s], zero outside the kernel.
    """
    np8 = _np_dt(mybir.dt.float8e4)
    nrows = 4 * FP8G7
    w = np.zeros((D, C, nrows, 32), np.float32)
    w[:, :, : KH - FP8_ROW0, :KH] = (
        weight[:, :, FP8_ROW0:KH, :].astype(np.float32) * W8SCALE
    )
    hi = w.astype(np8).astype(np.float32)
    lo = (w - hi).astype(np8).astype(np.float32)
    # [t, D, C, g8, u, vb, s] -> [u, c, g8, t, vb, s, d]
    tbl = np.stack([hi, lo], axis=0).reshape(2, D, C, FP8G7, 4, 8, 4)
    tbl = tbl.transpose(4, 2, 3, 0, 5, 6, 1).reshape(4 * C, FP8G7, 2, 8, 4 * D)
    return np.ascontiguousarray(tbl.astype(np8))


# ---------------------------------------------------------------------------
# FFT-hybrid path: 256-point DFT along H (dense matmuls), per-plane-group 1-D
# correlation along W (31 taps, block-diagonal complex matmuls), inverse DFT
# along H.  PE work is ~7x lower than the direct path (2240 vs 14464 matmuls
# per core).  Everything stays batch-sharded; no collectives.
#
# Plane ordering p = g*4 + slot over the 256 nontrivial rfft planes:
#   g=0:    [re0, re128, re64, im64]          (hf 0/128 are purely real)
#   g=1..63: [re_a, im_a, re_b, im_b] for consecutive pairs (a, b) drawn from
#            hfs [1..63, 65..127].
# Stage A:  Sh[p, c, w]   = sum_h  A[p, h] * sig[c, h, w]
# Stage B:  oh[(p,d), j]  = sum_v  L[g, v].T @ ShT_g[:, j+v]   (psum, 31 mms)
#           (bias folded into the DC plane: oh[p=0, d] += 256*bias[d])
# Stage C:  out[i, d, j]  = sum_p invF[i, p] * oh[p, d, j]
ALGO = os.environ.get("FFTCONV_ALGO", "fft")


def _plane_list():
    hfs = [h for h in range(1, 128) if h != 64]
    planes = [(0, "re"), (128, "re"), (64, "re"), (64, "im")]
    for i in range(0, len(hfs), 2):
        a, bb = hfs[i], hfs[i + 1]
        planes += [(a, "re"), (a, "im"), (bb, "re"), (bb, "im")]
    assert len(planes) == 256
    return planes


def _pack_fft_tables(weight: np.ndarray, bias: np.ndarray):
    """Host-side DFT/weight tables for the FFT-hybrid program (fp16)."""
    f16 = np.float16
    planes = _plane_list()
    h = np.arange(H)
    A = np.zeros((256, H), np.float64)
    for p, (hf, part) in enumerate(planes):
        ang = 2 * np.pi * hf * h / H
        A[p] = np.cos(ang) if part == "re" else -np.sin(ang)
    # lhsT[k=h, kc, mb, m=plane-local]
    aT = np.ascontiguousarray(
        A.T.reshape(2, 128, 2, 128).transpose(1, 0, 2, 3).astype(f16)
    )

    ii = np.arange(TH)
    invF = np.zeros((TH, 256), np.float64)
    for p, (hf, part) in enumerate(planes):
        ang = 2 * np.pi * hf * ii / H
        sc = (1.0 if hf in (0, 128) else 2.0) / H
        invF[:, p] = sc * (np.cos(ang) if part == "re" else -np.sin(ang))
    # lhsT[k=plane-local, kc, mb, m=i-local(113)]
    invT = np.ascontiguousarray(
        invF.T.reshape(2, 128, 2, 113).transpose(1, 0, 2, 3).astype(f16)
    )

    u = np.arange(KH)
    hf = np.arange(129)
    ph = np.exp(2j * np.pi * np.outer(u, hf) / H)  # [KH, 129]
    whc = np.einsum("dcuv,uf->dcvf", weight.astype(np.float64), ph)
    whr, whi = whc.real, whc.imag  # [D, C, KH, 129]
    L = np.zeros((64, KH, 128, 128), np.float32)
    for g in range(64):
        slots = planes[g * 4 : g * 4 + 4]
        loc = {pp: s for s, pp in enumerate(slots)}
        for hfv in sorted(set(f for f, _ in slots)):
            if hfv in (0, 128):
                s = loc[(hfv, "re")]
                L[g, :, s * 32 : s * 32 + 32, s * 32 : s * 32 + 32] = whr[
                    :, :, :, hfv
                ].transpose(2, 1, 0)
            else:
                sr, si = loc[(hfv, "re")], loc[(hfv, "im")]
                wr = whr[:, :, :, hfv].transpose(2, 1, 0)  # [v, c, d]
                wi = whi[:, :, :, hfv].transpose(2, 1, 0)
                L[g, :, sr * 32 : sr * 32 + 32, sr * 32 : sr * 32 + 32] = wr
                L[g, :, si * 32 : si * 32 + 32, sr * 32 : sr * 32 + 32] = -wi
                L[g, :, sr * 32 : sr * 32 + 32, si * 32 : si * 32 + 32] = wi
                L[g, :, si * 32 : si * 32 + 32, si * 32 : si * 32 + 32] = wr
    # Keep only the two 64x64 diagonal blocks (the rest of L is zero):
    # k 0..63 -> block0 (m 0..63), k 64..127 -> block1 (m 64..127).
    # [g, v, k, m] -> [g, k, v, m64]
    Lp = np.concatenate(
        [L[:, :, 0:64, 0:64], L[:, :, 64:128, 64:128]], axis=2
    )  # [g, v, 128, 64]
    lT = np.ascontiguousarray(Lp.transpose(0, 2, 1, 3).astype(f16))
    b256 = np.ascontiguousarray(
        (np.asarray(bias, np.float64) * H).astype(np.float32).reshape(D, 1)
    )
    return aT, lT, invT, b256


def build_program_fft():
    f16 = mybir.dt.float16
    f32 = mybir.dt.float32
    nc = bacc.Bacc(
        "TRN2",
        target_bir_lowering=False,
        debug=False,
        enable_asserts=False,
        num_devices=NCORES,
    )
    # signal pre-transposed on host to [b, h, c, w] so one DMA per
    # (b, kc, wq) loads [h=128, c, w-quarter] with matching dim order.
    sig_d = nc.dram_tensor("signal", [BPC, H, C, W], f16, kind="ExternalInput")
    a_d = nc.dram_tensor("aT", [128, 2, 2, 128], f16, kind="ExternalInput")
    l_d = nc.dram_tensor("lT", [64, 128, KH, 64], f16, kind="ExternalInput")
    inv_d = nc.dram_tensor("invT", [128, 2, 2, 113], f16, kind="ExternalInput")
    bias_d = nc.dram_tensor("bias256", [D, 1], f32, kind="ExternalInput")
    out_d = nc.dram_tensor("out", [BPC, D, TH, TW], f32, kind="ExternalOutput")
    IDENT = mybir.ActivationFunctionType.Identity

    SPITCH = 260  # sht w pitch: taps read cols v..v+228, v<=30 -> 259 used
    with tile.TileContext(nc) as tc:
        with (
            tc.tile_pool(name="const", bufs=1) as const_pool,
            tc.tile_pool(name="sht", bufs=1) as sht_pool,
            tc.tile_pool(name="ohT", bufs=1) as ohT_pool,
            tc.tile_pool(name="sig", bufs=1) as sig_pool,
            tc.tile_pool(name="stg", bufs=2) as stg_pool,
            tc.tile_pool(name="wl", bufs=6) as wl_pool,
            tc.tile_pool(name="oh", bufs=4) as oh_pool,
            tc.tile_pool(name="ob", bufs=4) as ob_pool,
            tc.tile_pool(name="psumA", bufs=1, space="PSUM") as psumA_pool,
            tc.tile_pool(name="psumB", bufs=6, space="PSUM") as psumB_pool,
            tc.tile_pool(name="psumC", bufs=1, space="PSUM") as psumC_pool,
        ):
            a_sb = const_pool.tile([128, 2, 2, 128], f16)
            nc.sync.dma_start(a_sb[:, :, :, :], a_d[:, :, :, :])
            inv_sb = const_pool.tile([128, 2, 2, 113], f16)
            nc.sync.dma_start(inv_sb[:, :, :, :], inv_d[:, :, :, :])
            bias_t = const_pool.tile([D, 1], f32)
            nc.sync.dma_start(bias_t[:, :], bias_d[:, :])

            # Single arena for all 64 group tiles: one scatter DMA per
            # (b, wq, mb) fills 32 groups at once.  Pad columns are zeroed
            # once (taps read cols up to 258; only psum columns j >= 226,
            # which stage C discards, see them).
            sht_ar = sht_pool.tile([128, 64, BPC, SPITCH], f16, tag="sht")
            nc.vector.memset(sht_ar[:, :, :, 256:SPITCH], 0.0)
            ohT = []
            for kc in range(2):
                ohT_kc = ohT_pool.tile([128, D, BPC, 229], f16, tag=f"ohT{kc}")
                ohT.append(ohT_kc)

            # ---- Stage A: H-DFT + scatter into per-group tiles ----
            for b in range(BPC):
                for wq in range(4):  # w in quarters of 64
                    sig_t = sig_pool.tile([128, 2, C, 64], f16, tag="sig")
                    for kc in range(2):
                        nc.sync.dma_start(
                            sig_t[:, kc, :, :],
                            sig_d[
                                b,
                                kc * 128 : (kc + 1) * 128,
                                :,
                                wq * 64 : (wq + 1) * 64,
                            ],
                        )
                    for mb in range(2):
                        stg = stg_pool.tile([128, C, 64], f16, tag="stg")
                        stgf = stg[:].rearrange("p c w -> p (c w)")
                        for ch in range(4):  # 8 c x 64 w = 512 per chunk
                            ps = psumA_pool.tile([128, 512], f32, tag="psA")
                            for kc in range(2):
                                nc.tensor.matmul(
                                    ps[:, :],
                                    lhsT=a_sb[:, kc, mb, :],
                                    rhs=sig_t[:, kc, 8 * ch : 8 * ch + 8, :].rearrange(
                                        "p c w -> p (c w)"
                                    ),
                                    start=(kc == 0),
                                    stop=(kc == 1),
                                )
                            nc.vector.tensor_scalar_add(
                                stgf[:, 512 * ch : 512 * (ch + 1)], ps[:, :], 0.0
                            )
                        for gl in range(32):
                            g = mb * 32 + gl
                            # dst partition dim stays whole so dep tracking
                            # is exact; src enumerates (slot, c, w).  Split
                            # across GPSIMD/ACT queues (both idle in stage
                            # A) — SP dispatch at ~0.8us each was the
                            # kernel-wide bottleneck.
                            eng = nc.gpsimd if gl % 2 == 0 else nc.scalar
                            eng.dma_start(
                                sht_ar[:, g, b, wq * 64 : (wq + 1) * 64],
                                stg[gl * 4 : (gl + 1) * 4, :, :],
                            )

            # ---- Stage B: 31-tap correlation per plane group ----
            # L is block-diagonal (2x 64x64); only the nonzero blocks are
            # shipped from HBM (halves the weight stream, which is the
            # stage-B bound).  The stationary is reassembled in persistent
            # SBUF buffers whose zero corners are set once.
            wlbufs = []
            for wi in range(6):
                wlb = wl_pool.tile([128, KH, 128], f16, tag="wl", name=f"wl{wi}")
                nc.vector.memset(wlb[0:64, :, 64:128], 0.0)
                nc.vector.memset(wlb[64:128, :, 0:64], 0.0)
                wlbufs.append(wlb)
            for g in range(64):
                wl = wlbufs[g % 6]
                nc.sync.dma_start(wl[0:64, :, 0:64], l_d[g, 0:64, :, :])
                nc.gpsimd.dma_start(wl[64:128, :, 64:128], l_d[g, 64:128, :, :])
                pB = psumB_pool.tile([128, BPC, 229], f32, tag="psB")
                for v in range(KH):
                    nc.tensor.matmul(
                        pB[:, :, :],
                        lhsT=wl[:, v, :],
                        rhs=sht_ar[:, g, :, v : v + 229],
                        start=(v == 0),
                        stop=(v == KH - 1),
                    )
                oh = oh_pool.tile([128, BPC, 229], f16, tag="oh")
                if g == 0:
                    nc.scalar.activation(
                        oh[0:32, :, :], pB[0:32, :, :], IDENT, bias=bias_t[:, :]
                    )
                    for s in range(1, 4):
                        nc.vector.tensor_scalar_add(
                            oh[s * 32 : (s + 1) * 32, :, :],
                            pB[s * 32 : (s + 1) * 32, :, :],
                            0.0,
                        )
                else:
                    nc.vector.tensor_scalar_add(oh[:, :, :], pB[:, :, :], 0.0)
                kc, pl = divmod(g * 4, 128)
                # src partition p = (slot, d) enumerates to match the dst
                # dims (slot-part, d, b, j).
                nc.scalar.dma_start(ohT[kc][pl : pl + 4, :, :, :], oh[:, :, :])

            # ---- Stage C: inverse H-DFT + store (bias already applied) ----
            for mb in range(2):
                for d in range(D):
                    pC = psumC_pool.tile([113, BPC, 229], f32, tag="psC")
                    for kc in range(2):
                        nc.tensor.matmul(
                            pC[:, :, :],
                            lhsT=inv_sb[:, kc, mb, :],
                            rhs=ohT[kc][:, d, :, :],
                            start=(kc == 0),
                            stop=(kc == 1),
                        )
                    ob = ob_pool.tile([113, BPC, TW], f32, tag="ob")
                    nc.vector.tensor_scalar_add(ob[:, :, :], pC[:, :, 0:226], 0.0)
                    for b in range(BPC):
                        nc.gpsimd.dma_start(
                            out_d[b, d, mb * 113 : (mb + 1) * 113, :],
                            ob[:, b, :],
                        )
    nc.compile()
    return nc


def make_in_maps_fft(signal, weight, bias):
    aT, lT, invT, b256 = _pack_fft_tables(np.asarray(weight), bias)
    # [b, c, h, w] -> [b, h, c, w]
    sig = np.asarray(signal).astype(np.float16, copy=False).transpose(0, 2, 1, 3)
    maps = []
    for c in range(NCORES):
        maps.append(
            {
                "signal": np.ascontiguousarray(sig[c * BPC : (c + 1) * BPC]),
                "aT": aT,
                "lT": lT,
                "invT": invT,
                "bias256": b256,
            }
        )
    return maps


_PROGRAM_CACHE: dict[str, object] = {}


def _get_program(dt_key: str):
    key = (ALGO, dt_key, WAVE, FP8G7, FP8V2)
    prog = _PROGRAM_CACHE.get(key)
    if prog is None:
        prog = build_program_fft() if ALGO == "fft" else build_program(dt_key)
        _PROGRAM_CACHE[key] = prog
    return prog


def make_in_maps(signal, weight, bias, dt_key: str = DT_KEY):
    if ALGO == "fft":
        return make_in_maps_fft(signal, weight, bias)
    wt_dt, sig_dt, _ = _DT_CONFIGS[dt_key]
    weight = np.asarray(weight)
    wT = pack_weights(weight, _np_dt(wt_dt))
    sig = np.asarray(signal).astype(_np_dt(sig_dt), copy=False)
    b2 = np.ascontiguousarray(np.asarray(bias, np.float32).reshape(D, 1))
    if FP8G7:
        wT8 = pack_weights8_v2(weight) if FP8V2 else pack_weights8(weight)
        sig8 = np.asarray(signal).astype(_np_dt(mybir.dt.float8e4))
    maps = []
    for c in range(NCORES):
        m = {
            "signal": np.ascontiguousarray(sig[c * BPC : (c + 1) * BPC]),
            "wT": wT,
            "bias": b2,
        }
        if FP8G7:
            m["sig8"] = np.ascontiguousarray(sig8[c * BPC : (c + 1) * BPC])
            m["wT8"] = wT8
        maps.append(m)
    return maps


class _Executor:
    """Cached jitted shard_map executor (re-jitting per call costs ~7 s).

    Outputs are fully written by the kernel each run, so the previous
    call's output buffers are donated as the next call's NEFF output
    operands (no fresh zero upload per call).
    """

    def __init__(self, nc):
        import jax
        from concourse.bass2jax import (
            _bass_exec_p,
            install_neuronx_cc_hook,
            partition_id_tensor,
        )
        from jax.sharding import Mesh, NamedSharding, PartitionSpec

        try:
            from jax.experimental.shard_map import shard_map
        except ImportError:
            from jax import shard_map

        install_neuronx_cc_hook()
        self.jax = jax
        part_name = nc.partition_id_tensor.name if nc.partition_id_tensor else None
        in_names, out_names, out_avals = [], [], []
        for alloc in nc.m.functions[0].allocations:
            if not isinstance(alloc, mybir.MemoryLocationSet):
                continue
            name = alloc.memorylocations[0].name
            if alloc.kind == "ExternalInput":
                if name != part_name:
                    in_names.append(name)
            elif alloc.kind == "ExternalOutput":
                out_names.append(name)
                out_avals.append(
                    jax.core.ShapedArray(
                        tuple(alloc.tensor_shape), mybir.dt.np(alloc.dtype)
                    )
                )
        self.in_names, self.out_names, self.out_avals = in_names, out_names, out_avals
        n_params = len(in_names)
        all_in = list(in_names) + list(out_names)
        if part_name is not None:
            all_in.append(part_name)

        def _body(*args):
            operands = list(args)
            if part_name is not None:
                operands.append(partition_id_tensor())
            return tuple(
                _bass_exec_p.bind(
                    *operands,
                    out_avals=tuple(out_avals),
                    in_names=tuple(all_in),
                    out_names=tuple(out_names),
                    lowering_input_output_aliases=(),
                    sim_require_finite=True,
                    sim_require_nnan=True,
                    nc=nc,
                )
            )

        devices = jax.devices()[:NCORES]
        mesh = Mesh(np.asarray(devices), ("core",))
        n_outs = len(out_names)
        self.fn = jax.jit(
            shard_map(
                _body,
                mesh=mesh,
                in_specs=(PartitionSpec("core"),) * (n_params + n_outs),
                out_specs=(PartitionSpec("core"),) * n_outs,
                check_rep=False,
            ),
            donate_argnums=tuple(range(n_params, n_params + n_outs)),
        )
        self.in_sharding = NamedSharding(mesh, PartitionSpec("core"))
        self.prev_outs = None

    def run(self, in_maps):
        jax = self.jax
        concat_in = [
            np.concatenate([np.asarray(m[n]) for m in in_maps], axis=0)
            for n in self.in_names
        ]
        dev_in = jax.device_put(concat_in, [self.in_sharding] * len(concat_in))
        outs = self.prev_outs
        if outs is None:
            outs = [
                np.zeros((NCORES * a.shape[0], *a.shape[1:]), a.dtype)
                for a in self.out_avals
            ]
        outs = self.fn(*dev_in, *outs)
        jax.block_until_ready(outs)
        host = {n: np.asarray(o) for n, o in zip(self.out_names, outs)}
        self.prev_outs = list(outs)
        return host


_EXECUTOR_CACHE: dict = {}


def _get_executor():
    key = (ALGO, DT_KEY, WAVE, FP8G7, FP8V2)
    ex = _EXECUTOR_CACHE.get(key)
    if ex is None:
        ex = _Executor(_get_program(DT_KEY))
        _EXECUTOR_CACHE[key] = ex
    return ex


def kernel(signal, weight, bias):
    in_maps = make_in_maps(signal, weight, bias, DT_KEY)
    try:
        host = _get_executor().run(in_maps)
        out_full = host["out"]
    except Exception:
        # Fallback: the stock (slower, re-jitting) execution path.
        nc = _get_program(DT_KEY)
        res = run_bass_kernel_spmd(nc, in_maps, list(range(NCORES)))
        out_full = np.concatenate(
            [res.results[c]["out"] for c in range(NCORES)], axis=0
        )
    out = out_full.reshape(B, D, TH, TW)
    return np.ascontiguousarray(out.astype(np.float32, copy=False))

